# revision 15
# baseline (speedup 1.0000x reference)
"""GCN (2-layer GCNConv + mean-pool + log_softmax) on 8 TRN2 NeuronCores.

One SPMD Bass kernel launch does all heavy work:
  phase 1: h'_i = dinv_i * (x_i @ W1)      per-core matmul over 98 row tiles
  phase 2: AllGather h' shards -> hfull    on-device collective (26 MB fp16)
  phase 3: acc[c] += h'[row_e] over edges  dma_gather (by source, 4 banks of
           25088 rows, int16 local idx) + dma_scatter_add (by dest),
           13 chunks x 8192 edges per bank, trash-row padding
  phase 4: h1 = relu(dinv*(acc + h'_local) + b1);  pooled += U_t^T @ h1_t
  output : pooled [128 graphs, 128 feat] partial per core

Host-side algebra that makes this possible:
  * GCN norm dinv[r]*dinv[c] factors into per-node scalings: scale rows of
    x@W1 by dinv once (sources), scale aggregated sums by dinv once (dests);
    the per-edge multiply disappears -> the SpMM is pure gather/scatter DMA.
  * Layer 2 + mean-pool collapse: pooled_sum[g] = sum_r U[r,g]*(h1[r]@W2)
    with U[r,g] = dinv[r]*(sum_{e:row=r} dinv[col_e]*[batch[col_e]=g] +
    dinv[r]*[batch[r]=g]) -- a dense [N,128] matrix built with one bincount.
    The second message-passing pass and pooling scatter are never executed.
"""

import os
import threading

import numpy as np

os.environ.setdefault("MYCRO_LOCAL_CACHE", "1")


def _warm_devices():
    # First contact with the axon-tunneled NeuronCores pays ~10-15 s of
    # device/terminal initialization regardless of payload size. Start it
    # in the background at import so it overlaps caller setup/preprocess.
    try:
        import jax
        ws = [jax.device_put(np.zeros(8, np.float32), d)
              for d in jax.devices()[:8]]
        for w in ws:
            w.block_until_ready()
    except Exception:
        pass


_WARM_THREAD = threading.Thread(target=_warm_devices, daemon=True)
_WARM_THREAD.start()

C = 8            # cores
NPR = 12500      # real nodes per core
NPP = 12544      # padded nodes per core (98 * 128)
TPC = NPP // 128  # 98 row tiles per core
D_IN = 768
KT = D_IN // 128  # 6 contraction tiles
NB = 4           # source banks
BANK = C * NPP // NB   # 25088 rows per bank (int16-addressable)
CH = 52          # chunks per (core, bank)
CW = 2048        # edges per chunk
SW = CW // 16    # idx columns per chunk
G = 128          # graphs
N = C * NPR      # 100000 nodes

_NC_CACHE = {}
LAST_RESULT = {}


def _build_nc():
    skip = set((os.environ.get("GCN_SKIP") or "").split(","))
    key = ("nc", tuple(sorted(skip)))
    if key in _NC_CACHE:
        return _NC_CACHE[key]
    import concourse.bacc as bacc
    import concourse.bass as bass
    import concourse.tile as tile
    from concourse import mybir

    f32 = mybir.dt.float32
    f16 = mybir.dt.float16
    i16 = mybir.dt.int16
    NT = C * NPP

    nc = bacc.Bacc("TRN2", target_bir_lowering=False, debug=False,
                   num_devices=C, dynamic_dma_scratch_size=131072)
    xT = nc.dram_tensor("xT", [KT, 128, NPP], f16, kind="ExternalInput")
    w1 = nc.dram_tensor("w1", [KT, 128, 128], f16, kind="ExternalInput")
    dinv = nc.dram_tensor("dinv", [128, TPC], f32, kind="ExternalInput")
    b1 = nc.dram_tensor("b1", [128, 128], f32, kind="ExternalInput")
    sidx = nc.dram_tensor("sidx", [NB, CH, 1, 16, SW], i16,
                          kind="ExternalInput")
    didx = nc.dram_tensor("didx", [NB, CH, 1, 16, SW], i16,
                          kind="ExternalInput")
    uu = nc.dram_tensor("uu", [TPC, 128, 128], f16, kind="ExternalInput")
    pooled = nc.dram_tensor("pooled", [128, 128], f32, kind="ExternalOutput")
    acc = nc.dram_tensor("acc", [NPP, 128], f16)
    hp = nc.dram_tensor("hp", [NPP, 128], f16)
    hfull = nc.dram_tensor("hfull", [NT, 128], f16)

    with tile.TileContext(nc) as tc:
        with (
            tc.tile_pool(name="wp", bufs=1) as wp,
            tc.tile_pool(name="cp", bufs=1) as cp,
            tc.tile_pool(name="xp", bufs=4) as xp,
            tc.tile_pool(name="hp_p", bufs=4) as hpp,
            tc.tile_pool(name="ps1", bufs=4, space=bass.MemorySpace.PSUM) as ps1,
            tc.tile_pool(name="gp", bufs=3) as gp,
            tc.tile_pool(name="ip", bufs=4) as ip,
            tc.tile_pool(name="ap4", bufs=4) as ap4,
            tc.tile_pool(name="up", bufs=4) as up,
            tc.tile_pool(name="ps2", bufs=1, space=bass.MemorySpace.PSUM) as ps2,
        ):
            wt = [wp.tile([128, 128], f16, name=f"wt{k}") for k in range(KT)]
            for k in range(KT):
                nc.sync.dma_start(wt[k][:, :], w1[k, :, :])
            dv = cp.tile([128, TPC], f32)
            nc.sync.dma_start(dv[:, :], dinv[:, :])
            b1t = cp.tile([128, 128], f32)
            nc.sync.dma_start(b1t[:, :], b1[:, :])
            zt = cp.tile([128, 128], f16)
            nc.vector.memset(zt[:, :], 0.0)
            for t in range(TPC):
                nc.sync.dma_start(acc[t * 128:(t + 1) * 128, :], zt[:, :])

            # phase 1
            for t in ([] if "1" in skip else range(TPC)):
                xt = [xp.tile([128, 128], f16, name=f"xt{k}")
                      for k in range(KT)]
                for k in range(KT):
                    nc.sync.dma_start(
                        xt[k][:, :], xT[k, :, t * 128:(t + 1) * 128])
                ps = ps1.tile([128, 128], f32)
                for k in range(KT):
                    nc.tensor.matmul(ps[:, :], xt[k][:, :], wt[k][:, :],
                                     start=(k == 0), stop=(k == KT - 1))
                ht = hpp.tile([128, 128], f16)
                nc.scalar.activation(ht[:, :], ps[:, :],
                                     mybir.ActivationFunctionType.Copy,
                                     scale=dv[:, t:t + 1])
                nc.sync.dma_start(hp[t * 128:(t + 1) * 128, :], ht[:, :])

            # phase 2
            if "2" not in skip:
              nc.gpsimd.collective_compute(
                "AllGather",
                mybir.AluOpType.bypass,
                replica_groups=[list(range(C))],
                ins=[hp[:, :]],
                outs=[hfull[:, :]],
              )

            # phase 3
            for b in ([] if "3" in skip else range(NB)):
                src = hfull[b * BANK:(b + 1) * BANK, :]
                for ci in range(CH):
                    st = ip.tile([128, SW], i16)
                    nc.scalar.dma_start(
                        st[:, :],
                        sidx[b, ci, :, :, :].broadcast_to([8, 16, SW]))
                    dt_ = ip.tile([128, SW], i16)
                    nc.sync.dma_start(
                        dt_[:, :],
                        didx[b, ci, :, :, :].broadcast_to([8, 16, SW]))
                    gt = gp.tile([128, CW // 128, 128], f16)
                    nc.gpsimd.dma_gather(
                        gt[:, :, :], src, st[:, :], CW, CW, 128,
                        single_packet=False)
                    nc.gpsimd.dma_scatter_add(
                        acc[:, :], gt[:, :, :], dt_[:, :], CW, CW, 128,
                        single_packet=False)

            # phase 4
            pooled_ps = ps2.tile([128, 128], f32)
            if "4" in skip:
                nc.tensor.matmul(pooled_ps[:, :], zt[:, :], zt[:, :],
                                 start=True, stop=True)
            for t in ([] if "4" in skip else range(TPC)):
                at = ap4.tile([128, 128], f16)
                nc.sync.dma_start(at[:, :], acc[t * 128:(t + 1) * 128, :])
                lt = ap4.tile([128, 128], f16)
                nc.scalar.dma_start(lt[:, :], hp[t * 128:(t + 1) * 128, :])
                s1 = ap4.tile([128, 128], f32)
                nc.vector.tensor_tensor(s1[:, :], at[:, :], lt[:, :],
                                        op=mybir.AluOpType.add)
                s2 = ap4.tile([128, 128], f32)
                nc.scalar.activation(s2[:, :], s1[:, :],
                                     mybir.ActivationFunctionType.Copy,
                                     scale=dv[:, t:t + 1])
                s3 = ap4.tile([128, 128], f32)
                nc.vector.tensor_tensor(s3[:, :], s2[:, :], b1t[:, :],
                                        op=mybir.AluOpType.add)
                h1 = ap4.tile([128, 128], f16)
                nc.scalar.activation(h1[:, :], s3[:, :],
                                     mybir.ActivationFunctionType.Relu)
                ut = up.tile([128, 128], f16)
                nc.sync.dma_start(ut[:, :], uu[t, :, :])
                nc.tensor.matmul(pooled_ps[:, :], ut[:, :], h1[:, :],
                                 start=(t == 0), stop=(t == TPC - 1))
            pt = up.tile([128, 128], f32)
            nc.vector.tensor_copy(pt[:, :], pooled_ps[:, :])
            nc.sync.dma_start(pooled[:, :], pt[:, :])

    nc.compile()
    _NC_CACHE[key] = nc
    return nc


def _wrap_idx(a, sw):
    """[n*16*sw] int16 -> [n, 16, sw]: i%16 partition, i//16 column;
    replication to 128 partitions happens on device (broadcast DMA)."""
    n = a.shape[0] // (16 * sw)
    return np.ascontiguousarray(a.reshape(n, sw, 16).transpose(0, 2, 1))


def _preprocess(x, row, col, bt, W1, b1):
    deg = (np.bincount(col, minlength=N) + 1).astype(np.float32)
    dinv = 1.0 / np.sqrt(deg)

    key = row.astype(np.int64) * G + bt[col]
    U = np.bincount(key, weights=dinv[col].astype(np.float64),
                    minlength=N * G).reshape(N, G)
    U[np.arange(N), bt] += dinv.astype(np.float64)
    U = (U * dinv[:, None]).astype(np.float32)

    core = col // NPR
    gsrc = (row // NPR) * NPP + (row % NPR)
    bank = gsrc // BANK
    sloc = (gsrc % BANK).astype(np.int16)
    dloc = (col % NPR).astype(np.int16)
    key2 = (core * NB + bank).astype(np.int16)
    order = np.argsort(key2, kind="stable")
    sloc, dloc = sloc[order], dloc[order]
    counts = np.bincount(key2, minlength=C * NB)
    assert counts.max() <= CH * CW, (counts.max(), CH * CW)
    offs = np.concatenate([[0], np.cumsum(counts)])

    slots = CH * CW
    trash = NPP - 1
    b1full = np.tile(np.asarray(b1, np.float32).reshape(1, 128), (128, 1))
    w1r = np.ascontiguousarray(np.asarray(W1, np.float32)
                               .reshape(KT, 128, 128)
                               .astype(np.float16))
    def _core_inputs(i):
        sfull = np.zeros((NB, slots), np.int16)
        dfull = np.full((NB, slots), trash, np.int16)
        for b in range(NB):
            k = i * NB + b
            n = counts[k]
            sfull[b, :n] = sloc[offs[k]:offs[k] + n]
            dfull[b, :n] = dloc[offs[k]:offs[k] + n]
        si = _wrap_idx(sfull.reshape(-1), SW)
        di = _wrap_idx(dfull.reshape(-1), SW)

        xpad = np.zeros((D_IN, NPP), np.float16)
        xpad[:, :NPR] = x[i * NPR:(i + 1) * NPR].T
        dpad = np.zeros(NPP, np.float32)
        dpad[:NPR] = dinv[i * NPR:(i + 1) * NPR]
        upad = np.zeros((NPP, G), np.float16)
        upad[:NPR] = U[i * NPR:(i + 1) * NPR]
        return {
            "xT": np.ascontiguousarray(xpad.reshape(KT, 128, NPP)),
            "w1": w1r,
            "dinv": np.ascontiguousarray(dpad.reshape(TPC, 128).T),
            "b1": b1full,
            "sidx": si.reshape(NB, CH, 1, 16, SW),
            "didx": di.reshape(NB, CH, 1, 16, SW),
            "uu": upad.reshape(TPC, 128, 128),
        }

    from concurrent.futures import ThreadPoolExecutor
    with ThreadPoolExecutor(max_workers=C) as ex:
        in_maps = list(ex.map(_core_inputs, range(C)))

    cnt = np.maximum(np.bincount(bt, minlength=G), 1.0).astype(np.float32)
    return in_maps, cnt


def _cpu_fallback(x, row, col, bt, W1, b1, W2, b2):
    deg = (np.bincount(col, minlength=N) + 1).astype(np.float32)
    dinv = 1.0 / np.sqrt(deg)
    hp = dinv[:, None] * (x @ W1)
    acc = np.zeros((N, 128), np.float32)
    try:
        from scipy.sparse import csr_matrix
        A = csr_matrix((np.ones(row.shape[0], np.float32), (col, row)),
                       shape=(N, N))
        acc = np.asarray(A @ hp, dtype=np.float32)
    except Exception:
        np.add.at(acc, col, hp[row])
    h1 = np.maximum(dinv[:, None] * (acc + hp) + b1, 0.0)
    z = h1 @ W2
    agg = np.zeros((N, z.shape[1]), np.float32)
    np.add.at(agg, col, dinv[col, None] * dinv[row, None] * z[row])
    agg += dinv[:, None] ** 2 * z
    sums = np.zeros((G, z.shape[1]), np.float32)
    np.add.at(sums, bt, agg)
    cnt = np.maximum(np.bincount(bt, minlength=G), 1.0).astype(np.float32)
    return sums / cnt[:, None] + b2, cnt


def _run_device(nc, in_maps):
    """Mirror of concourse.bass2jax.run_bass_via_pjrt with device-resident
    inputs and optional steady-state re-execution timing
    (GCN_HW_REPEATS=n -> LAST_RESULT["hw_ns"] = best repeat, measured on HW)."""
    import time
    import jax
    from jax.sharding import Mesh, PartitionSpec, NamedSharding
    from jax.experimental.shard_map import shard_map
    from concourse import mybir
    from concourse.bass2jax import (_bass_exec_p, install_neuronx_cc_hook,
                                    partition_id_tensor)

    install_neuronx_cc_hook()
    in_names, out_names, out_avals, zero_outs = [], [], [], []
    for alloc in nc.m.functions[0].allocations:
        if not isinstance(alloc, mybir.MemoryLocationSet):
            continue
        name = alloc.memorylocations[0].name
        if alloc.kind == "ExternalInput":
            if nc.partition_id_tensor is None or                     name != nc.partition_id_tensor.name:
                in_names.append(name)
        elif alloc.kind == "ExternalOutput":
            shape = tuple(alloc.tensor_shape)
            dtype = mybir.dt.np(alloc.dtype)
            out_names.append(name)
            out_avals.append(jax.core.ShapedArray(shape, dtype))
            zero_outs.append(np.zeros(shape, dtype))
    n_params = len(in_names)
    n_outs = len(out_avals)
    all_names = in_names + out_names
    partition_name = (nc.partition_id_tensor.name
                      if nc.partition_id_tensor else None)
    if partition_name is not None:
        all_names.append(partition_name)
    donate = tuple(range(n_params, n_params + n_outs))

    def _body(*args):
        operands = list(args)
        if partition_name is not None:
            operands.append(partition_id_tensor())
        return tuple(_bass_exec_p.bind(
            *operands,
            out_avals=tuple(out_avals),
            in_names=tuple(all_names),
            out_names=tuple(out_names),
            lowering_input_output_aliases=(),
            sim_require_finite=True,
            sim_require_nnan=True,
            nc=nc,
        ))

    devices = jax.devices()[:C]
    mesh = Mesh(np.asarray(devices), ("core",))
    spec = NamedSharding(mesh, PartitionSpec("core"))
    sharded = jax.jit(
        shard_map(_body, mesh=mesh,
                  in_specs=(PartitionSpec("core"),) * (n_params + n_outs),
                  out_specs=(PartitionSpec("core"),) * n_outs,
                  check_rep=False),
        donate_argnums=donate, keep_unused=True)

    t0 = time.time()
    dev_in = []
    for i, name in enumerate(in_names):
        arrs = [jax.device_put(in_maps[c][name], devices[c])
                for c in range(C)]
        gshape = (C * arrs[0].shape[0], *arrs[0].shape[1:])
        dev_in.append(jax.make_array_from_single_device_arrays(
            gshape, spec, arrs))
    for a in dev_in:
        a.block_until_ready()
    t1 = time.time()

    def one_exec():
        zs = [jax.device_put(
            np.zeros((C * z.shape[0], *z.shape[1:]), z.dtype), spec)
            for z in zero_outs]
        ts = time.time()
        outs = sharded(*dev_in, *zs)
        for o in outs:
            o.block_until_ready()
        return outs, time.time() - ts

    out_arrs, texec = one_exec()
    t2 = time.time()
    if os.environ.get("GCN_TIMING"):
        print(f"[timing] upload {t1-t0:.2f}s exec+compile {t2-t1:.2f}s",
              flush=True)
    reps = int(os.environ.get("GCN_HW_REPEATS", "0"))
    if reps:
        times = []
        for _ in range(reps):
            out_arrs, dt = one_exec()
            times.append(dt)
        LAST_RESULT["hw_ns"] = int(min(times) * 1e9)
        if os.environ.get("GCN_TIMING"):
            print(f"[timing] exec repeats: "
                  f"{[f'{x*1e3:.1f}ms' for x in times]}", flush=True)

    out_np = [np.asarray(a) for a in out_arrs]
    return [
        {name: out_np[i].reshape(C, *out_avals[i].shape)[c]
         for i, name in enumerate(out_names)}
        for c in range(C)
    ]


def kernel(x, edge_index, batch, W1, b1, W2, b2):
    x = np.asarray(x, np.float32)
    ei = np.asarray(edge_index)
    row = np.ascontiguousarray(ei[0]).astype(np.int32)
    col = np.ascontiguousarray(ei[1]).astype(np.int32)
    bt = np.asarray(batch).astype(np.int32)
    W2 = np.asarray(W2, np.float32)
    b2 = np.asarray(b2, np.float32)

    import time as _time
    _dbg = os.environ.get("GCN_TIMING")
    try:
        t0 = _time.time()
        in_maps, cnt = _preprocess(x, row, col, bt, W1, b1)
        t1 = _time.time()
        nc = _build_nc()
        t2 = _time.time()
        results = _run_device(nc, in_maps)
        t3 = _time.time()
        if _dbg:
            print(f"[timing] preprocess {t1-t0:.2f}s build {t2-t1:.2f}s "
                  f"run {t3-t2:.2f}s", flush=True)
        P = np.sum([np.asarray(results[i]["pooled"]) for i in range(C)],
                   axis=0)
        pooled = (P @ W2) / cnt[:, None] + b2
    except Exception:
        import traceback
        traceback.print_exc()
        pooled, cnt = _cpu_fallback(x, row, col, bt,
                                    np.asarray(W1, np.float32),
                                    np.asarray(b1, np.float32), W2, b2)

    m = pooled.max(axis=1, keepdims=True)
    ls = m + np.log(np.exp(pooled - m).sum(axis=1, keepdims=True))
    return (pooled - ls).astype(np.float32)


# revision 16
# speedup vs baseline: 1.6971x; 1.6971x over previous
"""GCN (2-layer GCNConv + mean-pool + log_softmax) on 8 TRN2 NeuronCores.

One SPMD Bass kernel launch does all heavy work:
  phase 1: h'_i = dinv_i * (x_i @ W1)      per-core matmul over 98 row tiles
  phase 2: AllGather h' shards -> hfull    on-device collective (26 MB fp16)
  phase 3: acc[c] += h'[row_e] over edges  dma_gather (by source, 4 banks of
           25088 rows, int16 local idx) + dma_scatter_add (by dest),
           13 chunks x 8192 edges per bank, trash-row padding
  phase 4: h1 = relu(dinv*(acc + h'_local) + b1);  pooled += U_t^T @ h1_t
  output : pooled [128 graphs, 128 feat] partial per core

Host-side algebra that makes this possible:
  * GCN norm dinv[r]*dinv[c] factors into per-node scalings: scale rows of
    x@W1 by dinv once (sources), scale aggregated sums by dinv once (dests);
    the per-edge multiply disappears -> the SpMM is pure gather/scatter DMA.
  * Layer 2 + mean-pool collapse: pooled_sum[g] = sum_r U[r,g]*(h1[r]@W2)
    with U[r,g] = dinv[r]*(sum_{e:row=r} dinv[col_e]*[batch[col_e]=g] +
    dinv[r]*[batch[r]=g]) -- a dense [N,128] matrix built with one bincount.
    The second message-passing pass and pooling scatter are never executed.
"""

import os
import threading

import numpy as np

os.environ.setdefault("MYCRO_LOCAL_CACHE", "1")


def _warm_devices():
    # First contact with the axon-tunneled NeuronCores pays ~10-15 s of
    # device/terminal initialization regardless of payload size. Start it
    # in the background at import so it overlaps caller setup/preprocess.
    try:
        import jax
        ws = [jax.device_put(np.zeros(8, np.float32), d)
              for d in jax.devices()[:8]]
        for w in ws:
            w.block_until_ready()
    except Exception:
        pass


_WARM_THREAD = threading.Thread(target=_warm_devices, daemon=True)
_WARM_THREAD.start()

C = 8            # cores
NPR = 12500      # real nodes per core
NPP = 12544      # padded nodes per core (98 * 128)
TPC = NPP // 128  # 98 row tiles per core
D_IN = 768
KT = D_IN // 128  # 6 contraction tiles
NB = 4           # source banks
BANK = C * NPP // NB   # 25088 rows per bank (int16-addressable)
CH = 52          # chunks per (core, bank)
CW = 2048        # edges per chunk
SW = CW // 16    # idx columns per chunk
G = 128          # graphs
N = C * NPR      # 100000 nodes

_NC_CACHE = {}
LAST_RESULT = {}


def _build_nc():
    skip = set((os.environ.get("GCN_SKIP") or "").split(","))
    key = ("nc", tuple(sorted(skip)))
    if key in _NC_CACHE:
        return _NC_CACHE[key]
    import concourse.bacc as bacc
    import concourse.bass as bass
    import concourse.tile as tile
    from concourse import mybir

    f32 = mybir.dt.float32
    f16 = mybir.dt.float16
    i16 = mybir.dt.int16
    NT = C * NPP

    nc = bacc.Bacc("TRN2", target_bir_lowering=False, debug=False,
                   num_devices=C, dynamic_dma_scratch_size=65536)
    xT = nc.dram_tensor("xT", [KT, 128, NPP], f16, kind="ExternalInput")
    w1 = nc.dram_tensor("w1", [KT, 128, 128], f16, kind="ExternalInput")
    dinv = nc.dram_tensor("dinv", [128, TPC], f32, kind="ExternalInput")
    b1 = nc.dram_tensor("b1", [128, 128], f32, kind="ExternalInput")
    sidx = nc.dram_tensor("sidx", [NB, CH, 1, 16, SW], i16,
                          kind="ExternalInput")
    didx = nc.dram_tensor("didx", [NB, CH, 1, 16, SW], i16,
                          kind="ExternalInput")
    uu = nc.dram_tensor("uu", [TPC, 128, 128], f16, kind="ExternalInput")
    pooled = nc.dram_tensor("pooled", [128, 128], f32, kind="ExternalOutput")
    acc = nc.dram_tensor("acc", [NPP, 128], f16)
    hp = nc.dram_tensor("hp", [NPP, 128], f16)
    hfull = nc.dram_tensor("hfull", [NT, 128], f16)

    with tile.TileContext(nc) as tc:
        with (
            tc.tile_pool(name="wp", bufs=1) as wp,
            tc.tile_pool(name="cp", bufs=1) as cp,
            tc.tile_pool(name="xp", bufs=4) as xp,
            tc.tile_pool(name="hp_p", bufs=4) as hpp,
            tc.tile_pool(name="ps1", bufs=4, space=bass.MemorySpace.PSUM) as ps1,
            tc.tile_pool(name="gp", bufs=3) as gp,
            tc.tile_pool(name="ip", bufs=2) as ip,
            tc.tile_pool(name="ap4", bufs=4) as ap4,
            tc.tile_pool(name="up", bufs=4) as up,
            tc.tile_pool(name="ps2", bufs=1, space=bass.MemorySpace.PSUM) as ps2,
        ):
            wt = [wp.tile([128, 128], f16, name=f"wt{k}") for k in range(KT)]
            for k in range(KT):
                nc.sync.dma_start(wt[k][:, :], w1[k, :, :])
            dv = cp.tile([128, TPC], f32)
            nc.sync.dma_start(dv[:, :], dinv[:, :])
            b1t = cp.tile([128, 128], f32)
            nc.sync.dma_start(b1t[:, :], b1[:, :])
            zt = cp.tile([128, 128], f16)
            nc.vector.memset(zt[:, :], 0.0)
            for t in range(TPC):
                nc.sync.dma_start(acc[t * 128:(t + 1) * 128, :], zt[:, :])

            # phase 1
            for t in ([] if "1" in skip else range(TPC)):
                xt = xp.tile([128, KT, 128], f16)
                nc.sync.dma_start(
                    xt[:, :, :],
                    xT[:, :, t * 128:(t + 1) * 128].transpose([1, 0, 2]))
                ps = ps1.tile([128, 128], f32)
                for k in range(KT):
                    nc.tensor.matmul(ps[:, :], xt[:, k, :], wt[k][:, :],
                                     start=(k == 0), stop=(k == KT - 1))
                ht = hpp.tile([128, 128], f16)
                nc.scalar.activation(ht[:, :], ps[:, :],
                                     mybir.ActivationFunctionType.Copy,
                                     scale=dv[:, t:t + 1])
                nc.sync.dma_start(hp[t * 128:(t + 1) * 128, :], ht[:, :])

            # phase 2
            if "2" not in skip:
              nc.gpsimd.collective_compute(
                "AllGather",
                mybir.AluOpType.bypass,
                replica_groups=[list(range(C))],
                ins=[hp[:, :]],
                outs=[hfull[:, :]],
              )

            # phase 3
            for b in ([] if "3" in skip else range(NB)):
                src = hfull[b * BANK:(b + 1) * BANK, :]
                st = ip.tile([128, CH, SW], i16)
                dt_ = ip.tile([128, CH, SW], i16)
                for r in range(8):
                    nc.scalar.dma_start(
                        st[16 * r:16 * (r + 1), :, :],
                        sidx[b, :, 0, :, :].transpose([1, 0, 2]))
                    nc.sync.dma_start(
                        dt_[16 * r:16 * (r + 1), :, :],
                        didx[b, :, 0, :, :].transpose([1, 0, 2]))
                for ci in range(CH):
                    gt = gp.tile([128, CW // 128, 128], f16)
                    nc.gpsimd.dma_gather(
                        gt[:, :, :], src, st[:, ci, :], CW, CW, 128,
                        single_packet=False)
                    nc.gpsimd.dma_scatter_add(
                        acc[:, :], gt[:, :, :], dt_[:, ci, :], CW, CW, 128,
                        single_packet=False)

            # phase 4
            pooled_ps = ps2.tile([128, 128], f32)
            if "4" in skip:
                nc.tensor.matmul(pooled_ps[:, :], zt[:, :], zt[:, :],
                                 start=True, stop=True)
            for t in ([] if "4" in skip else range(TPC)):
                at = ap4.tile([128, 128], f16)
                nc.sync.dma_start(at[:, :], acc[t * 128:(t + 1) * 128, :])
                lt = ap4.tile([128, 128], f16)
                nc.scalar.dma_start(lt[:, :], hp[t * 128:(t + 1) * 128, :])
                s1 = ap4.tile([128, 128], f32)
                nc.vector.tensor_tensor(s1[:, :], at[:, :], lt[:, :],
                                        op=mybir.AluOpType.add)
                s2 = ap4.tile([128, 128], f32)
                nc.scalar.activation(s2[:, :], s1[:, :],
                                     mybir.ActivationFunctionType.Copy,
                                     scale=dv[:, t:t + 1])
                s3 = ap4.tile([128, 128], f32)
                nc.vector.tensor_tensor(s3[:, :], s2[:, :], b1t[:, :],
                                        op=mybir.AluOpType.add)
                h1 = ap4.tile([128, 128], f16)
                nc.scalar.activation(h1[:, :], s3[:, :],
                                     mybir.ActivationFunctionType.Relu)
                ut = up.tile([128, 128], f16)
                nc.sync.dma_start(ut[:, :], uu[t, :, :])
                nc.tensor.matmul(pooled_ps[:, :], ut[:, :], h1[:, :],
                                 start=(t == 0), stop=(t == TPC - 1))
            pt = up.tile([128, 128], f32)
            nc.vector.tensor_copy(pt[:, :], pooled_ps[:, :])
            nc.sync.dma_start(pooled[:, :], pt[:, :])

    nc.compile()
    _NC_CACHE[key] = nc
    return nc


def _wrap_idx(a, sw):
    """[n*16*sw] int16 -> [n, 16, sw]: i%16 partition, i//16 column;
    replication to 128 partitions happens on device (broadcast DMA)."""
    n = a.shape[0] // (16 * sw)
    return np.ascontiguousarray(a.reshape(n, sw, 16).transpose(0, 2, 1))


def _preprocess(x, row, col, bt, W1, b1):
    deg = (np.bincount(col, minlength=N) + 1).astype(np.float32)
    dinv = 1.0 / np.sqrt(deg)

    key = row.astype(np.int64) * G + bt[col]
    U = np.bincount(key, weights=dinv[col].astype(np.float64),
                    minlength=N * G).reshape(N, G)
    U[np.arange(N), bt] += dinv.astype(np.float64)
    U = (U * dinv[:, None]).astype(np.float32)

    core = col // NPR
    gsrc = (row // NPR) * NPP + (row % NPR)
    bank = gsrc // BANK
    sloc = (gsrc % BANK).astype(np.int16)
    dloc = (col % NPR).astype(np.int16)
    key2 = (core * NB + bank).astype(np.int16)
    order = np.argsort(key2, kind="stable")
    sloc, dloc = sloc[order], dloc[order]
    counts = np.bincount(key2, minlength=C * NB)
    assert counts.max() <= CH * CW, (counts.max(), CH * CW)
    offs = np.concatenate([[0], np.cumsum(counts)])

    slots = CH * CW
    trash = NPP - 1
    b1full = np.tile(np.asarray(b1, np.float32).reshape(1, 128), (128, 1))
    w1r = np.ascontiguousarray(np.asarray(W1, np.float32)
                               .reshape(KT, 128, 128)
                               .astype(np.float16))
    def _core_inputs(i):
        sfull = np.zeros((NB, slots), np.int16)
        dfull = np.full((NB, slots), trash, np.int16)
        for b in range(NB):
            k = i * NB + b
            n = counts[k]
            sfull[b, :n] = sloc[offs[k]:offs[k] + n]
            dfull[b, :n] = dloc[offs[k]:offs[k] + n]
        si = _wrap_idx(sfull.reshape(-1), SW)
        di = _wrap_idx(dfull.reshape(-1), SW)

        xpad = np.zeros((D_IN, NPP), np.float16)
        xpad[:, :NPR] = x[i * NPR:(i + 1) * NPR].T
        dpad = np.zeros(NPP, np.float32)
        dpad[:NPR] = dinv[i * NPR:(i + 1) * NPR]
        upad = np.zeros((NPP, G), np.float16)
        upad[:NPR] = U[i * NPR:(i + 1) * NPR]
        return {
            "xT": np.ascontiguousarray(xpad.reshape(KT, 128, NPP)),
            "w1": w1r,
            "dinv": np.ascontiguousarray(dpad.reshape(TPC, 128).T),
            "b1": b1full,
            "sidx": si.reshape(NB, CH, 1, 16, SW),
            "didx": di.reshape(NB, CH, 1, 16, SW),
            "uu": upad.reshape(TPC, 128, 128),
        }

    from concurrent.futures import ThreadPoolExecutor
    with ThreadPoolExecutor(max_workers=C) as ex:
        in_maps = list(ex.map(_core_inputs, range(C)))

    cnt = np.maximum(np.bincount(bt, minlength=G), 1.0).astype(np.float32)
    return in_maps, cnt


def _cpu_fallback(x, row, col, bt, W1, b1, W2, b2):
    deg = (np.bincount(col, minlength=N) + 1).astype(np.float32)
    dinv = 1.0 / np.sqrt(deg)
    hp = dinv[:, None] * (x @ W1)
    acc = np.zeros((N, 128), np.float32)
    try:
        from scipy.sparse import csr_matrix
        A = csr_matrix((np.ones(row.shape[0], np.float32), (col, row)),
                       shape=(N, N))
        acc = np.asarray(A @ hp, dtype=np.float32)
    except Exception:
        np.add.at(acc, col, hp[row])
    h1 = np.maximum(dinv[:, None] * (acc + hp) + b1, 0.0)
    z = h1 @ W2
    agg = np.zeros((N, z.shape[1]), np.float32)
    np.add.at(agg, col, dinv[col, None] * dinv[row, None] * z[row])
    agg += dinv[:, None] ** 2 * z
    sums = np.zeros((G, z.shape[1]), np.float32)
    np.add.at(sums, bt, agg)
    cnt = np.maximum(np.bincount(bt, minlength=G), 1.0).astype(np.float32)
    return sums / cnt[:, None] + b2, cnt


def _run_device(nc, in_maps):
    """Mirror of concourse.bass2jax.run_bass_via_pjrt with device-resident
    inputs and optional steady-state re-execution timing
    (GCN_HW_REPEATS=n -> LAST_RESULT["hw_ns"] = best repeat, measured on HW)."""
    import time
    import jax
    from jax.sharding import Mesh, PartitionSpec, NamedSharding
    from jax.experimental.shard_map import shard_map
    from concourse import mybir
    from concourse.bass2jax import (_bass_exec_p, install_neuronx_cc_hook,
                                    partition_id_tensor)

    install_neuronx_cc_hook()
    in_names, out_names, out_avals, zero_outs = [], [], [], []
    for alloc in nc.m.functions[0].allocations:
        if not isinstance(alloc, mybir.MemoryLocationSet):
            continue
        name = alloc.memorylocations[0].name
        if alloc.kind == "ExternalInput":
            if nc.partition_id_tensor is None or                     name != nc.partition_id_tensor.name:
                in_names.append(name)
        elif alloc.kind == "ExternalOutput":
            shape = tuple(alloc.tensor_shape)
            dtype = mybir.dt.np(alloc.dtype)
            out_names.append(name)
            out_avals.append(jax.core.ShapedArray(shape, dtype))
            zero_outs.append(np.zeros(shape, dtype))
    n_params = len(in_names)
    n_outs = len(out_avals)
    all_names = in_names + out_names
    partition_name = (nc.partition_id_tensor.name
                      if nc.partition_id_tensor else None)
    if partition_name is not None:
        all_names.append(partition_name)
    donate = tuple(range(n_params, n_params + n_outs))

    def _body(*args):
        operands = list(args)
        if partition_name is not None:
            operands.append(partition_id_tensor())
        return tuple(_bass_exec_p.bind(
            *operands,
            out_avals=tuple(out_avals),
            in_names=tuple(all_names),
            out_names=tuple(out_names),
            lowering_input_output_aliases=(),
            sim_require_finite=True,
            sim_require_nnan=True,
            nc=nc,
        ))

    devices = jax.devices()[:C]
    mesh = Mesh(np.asarray(devices), ("core",))
    spec = NamedSharding(mesh, PartitionSpec("core"))
    sharded = jax.jit(
        shard_map(_body, mesh=mesh,
                  in_specs=(PartitionSpec("core"),) * (n_params + n_outs),
                  out_specs=(PartitionSpec("core"),) * n_outs,
                  check_rep=False),
        donate_argnums=donate, keep_unused=True)

    t0 = time.time()
    dev_in = []
    for i, name in enumerate(in_names):
        arrs = [jax.device_put(in_maps[c][name], devices[c])
                for c in range(C)]
        gshape = (C * arrs[0].shape[0], *arrs[0].shape[1:])
        dev_in.append(jax.make_array_from_single_device_arrays(
            gshape, spec, arrs))
    for a in dev_in:
        a.block_until_ready()
    t1 = time.time()

    def one_exec():
        zs = [jax.device_put(
            np.zeros((C * z.shape[0], *z.shape[1:]), z.dtype), spec)
            for z in zero_outs]
        ts = time.time()
        outs = sharded(*dev_in, *zs)
        for o in outs:
            o.block_until_ready()
        return outs, time.time() - ts

    out_arrs, texec = one_exec()
    t2 = time.time()
    if os.environ.get("GCN_TIMING"):
        print(f"[timing] upload {t1-t0:.2f}s exec+compile {t2-t1:.2f}s",
              flush=True)
    reps = int(os.environ.get("GCN_HW_REPEATS", "0"))
    if reps:
        times = []
        for _ in range(reps):
            out_arrs, dt = one_exec()
            times.append(dt)
        LAST_RESULT["hw_ns"] = int(min(times) * 1e9)
        if os.environ.get("GCN_TIMING"):
            print(f"[timing] exec repeats: "
                  f"{[f'{x*1e3:.1f}ms' for x in times]}", flush=True)

    out_np = [np.asarray(a) for a in out_arrs]
    return [
        {name: out_np[i].reshape(C, *out_avals[i].shape)[c]
         for i, name in enumerate(out_names)}
        for c in range(C)
    ]


def kernel(x, edge_index, batch, W1, b1, W2, b2):
    x = np.asarray(x, np.float32)
    ei = np.asarray(edge_index)
    row = np.ascontiguousarray(ei[0]).astype(np.int32)
    col = np.ascontiguousarray(ei[1]).astype(np.int32)
    bt = np.asarray(batch).astype(np.int32)
    W2 = np.asarray(W2, np.float32)
    b2 = np.asarray(b2, np.float32)

    import time as _time
    _dbg = os.environ.get("GCN_TIMING")
    try:
        t0 = _time.time()
        in_maps, cnt = _preprocess(x, row, col, bt, W1, b1)
        t1 = _time.time()
        nc = _build_nc()
        t2 = _time.time()
        results = _run_device(nc, in_maps)
        t3 = _time.time()
        if _dbg:
            print(f"[timing] preprocess {t1-t0:.2f}s build {t2-t1:.2f}s "
                  f"run {t3-t2:.2f}s", flush=True)
        P = np.sum([np.asarray(results[i]["pooled"]) for i in range(C)],
                   axis=0)
        pooled = (P @ W2) / cnt[:, None] + b2
    except Exception:
        import traceback
        traceback.print_exc()
        pooled, cnt = _cpu_fallback(x, row, col, bt,
                                    np.asarray(W1, np.float32),
                                    np.asarray(b1, np.float32), W2, b2)

    m = pooled.max(axis=1, keepdims=True)
    ls = m + np.log(np.exp(pooled - m).sum(axis=1, keepdims=True))
    return (pooled - ls).astype(np.float32)
